# revision 30
# baseline (speedup 1.0000x reference)
"""FESTGCN Trainium2 kernel v5: 8-core SPMD Bass/Tile implementation.

Algorithm (reference semantics, validated in sim_v2.py at ~5e-3):
  For t in 0..9:
    M_t = dtw * (ceil|td| > 9-t) + (spec_lap + I)       [t=9: host-folded
          (2/3)(dtw*(td>0) + spec_lap + I + laplacian), no mask]
    S1 += M_t^T-block @ c1_t      c1_t = [x_t | h_t]   (inputs only)
    r_t = sigmoid(0.5*S1@W1 + (t+1)b1) for r-half nodes (0..2047)
    S2 += M_t^T-block @ c2_t      c2_t = [x_t | r_t*h_t]
  u = sigmoid(...)[u-half] at t=9 ; c = tanh(0.5*S2@W2 + 10 b2)
  out = u*h_9 + (1-u)*c

Sharding: interleaved row blocks (core c owns nodes [c*256,(c+1)*256)
u [2048+c*256, ...)), so m-tiles 0,1 are r-nodes (per-step sigmoid +
AllGather payload) and 2,3 are u-nodes (sigmoid at t=9 only). conv1
depends only on inputs so it runs ahead; conv2 is emitted with a
2-step lag to hide AllGather latency; rl gathers ride the second
HWDGE ring (scalar) to avoid SP-FIFO head-of-line blocking.

Masks are host-precomputed fp16 and streamed from DRAM (memory
regime). AG payloads are partition-major ([p, j2-block, b, f], where
gathered hnode = (4*rank+j2)*128+p), so producer writes, collective
shards, and per-rank consumer reads are all contiguous-chunk DMAs.
"""

import numpy as np

import concourse.bacc as bacc
import concourse.mybir as mybir
import concourse.tile as tile
from concourse.bass_utils import run_bass_kernel_spmd

B, T, N, H = 4, 10, 4096, 32
NC = 8
HB = 256                 # nodes per half-block per core
RPC = 2 * HB             # 512 owned rows per core
NG = 8                   # contraction groups (4 n-tiles each)
NTPG = 4
MT = 4
F1 = B * (H + 1)         # 132
F4 = NTPG * F1           # 528
BH = B * H               # 128
f32 = mybir.dt.float32
f16 = mybir.dt.float16
Alu = mybir.AluOpType
Act = mybir.ActivationFunctionType
CORES = list(range(NC))


def _build_nc():
    nc = bacc.Bacc(
        "TRN2",
        target_bir_lowering=False,
        debug=False,
        enable_asserts=True,
        num_devices=NC,
    )
    maskT = nc.dram_tensor("maskT", [T, N, RPC], f16, kind="ExternalInput").ap()
    # states pre-gathered per step: [T, jj(16), p(128), j2(2)*BH]
    stN = nc.dram_tensor("stN", [T, 16, 128, 2 * BH], f16,
                         kind="ExternalInput").ap()
    xN = nc.dram_tensor("xN", [128, 32 * T * B], f16, kind="ExternalInput").ap()
    w1h = nc.dram_tensor("w1h", [H + 1, 2 * H], f32, kind="ExternalInput").ap()
    w2h = nc.dram_tensor("w2h", [H + 1, H], f32, kind="ExternalInput").ap()
    houtN = nc.dram_tensor("houtN", [N, BH], f32, kind="ExternalOutput").ap()

    with tile.TileContext(nc) as tc:
        with (
            tc.tile_pool(name="xp", bufs=1) as xp,
            tc.tile_pool(name="hp", bufs=4) as hp,
            tc.tile_pool(name="rlp", bufs=2) as rlp,
            tc.tile_pool(name="mp", bufs=26) as mp,
            tc.tile_pool(name="cp", bufs=8) as cp,
            tc.tile_pool(name="accp", bufs=2) as accp,
            tc.tile_pool(name="wk", bufs=2) as wk,
            tc.tile_pool(name="sm", bufs=1) as sm,
            tc.tile_pool(name="z1p", bufs=2, space="PSUM") as z1p,
            tc.tile_pool(name="z2p", bufs=1, space="PSUM") as z2p,
            tc.tile_pool(name="tpzp", bufs=1, space="PSUM") as tpzp,
            tc.tile_pool(name="g1p", bufs=1, space="PSUM") as g1p,
            tc.tile_pool(name="dramp", bufs=1, space="DRAM") as dramp,
        ):
            # partition-major r_t payloads: shard [p, j2(4)*BH]
            agsrc = [
                dramp.tile([128, 4 * BH], f16, tag=f"agsrc{t}", name=f"agsrc{t}")
                for t in range(T)
            ]
            agdst = [
                dramp.tile([NC, 128, 4 * BH], f16, tag=f"agdst{t}",
                           name=f"agdst{t}", addr_space="Shared")
                for t in range(T)
            ]
            usrc = dramp.tile([128, 4 * BH], f16, tag="usrc", name="usrc")
            udst = dramp.tile([NC, 128, 4 * BH], f16, tag="udst",
                              name="udst", addr_space="Shared")
            # c payload: [p, (c mt0..3)*BH]
            cusrc = dramp.tile([128, 4 * BH], f16, tag="cusrc", name="cusrc")
            cudst = dramp.tile([NC, 128, 4 * BH], f16, tag="cudst",
                               name="cudst", addr_space="Shared")

            # ---------------- prologue ----------------
            iota_i = wk.tile([128, 128], mybir.dt.int32, tag="iota", bufs=1)
            nc.gpsimd.iota(iota_i[:], pattern=[[1, 128]], base=0,
                           channel_multiplier=-1)
            ident = sm.tile([128, 128], f32, tag="ident")
            nc.vector.tensor_scalar(ident[:], iota_i[:], 0, None,
                                    op0=Alu.is_equal)

            xall = xp.tile([128, 32 * T * B], f16, tag="xall")
            nc.sync.dma_start(xall[:], xN[:])

            s1 = [accp.tile([128, F1], f32, tag=f"s1_{mt}", name=f"s1_{mt}")
                  for mt in range(MT)]
            s2 = [accp.tile([128, F1], f32, tag=f"s2_{mt}", name=f"s2_{mt}")
                  for mt in range(MT)]
            for mt in range(MT):
                nc.vector.memset(s1[mt][:], 0.0)
                nc.vector.memset(s2[mt][:], 0.0)

            hcur = [None] * T
            mtiles = [None] * T
            w1s = sm.tile([H + 1, 2 * H], f32, tag="w1s")
            w2s = sm.tile([H + 1, H], f32, tag="w2s")
            biasc = sm.tile([128, T], f32, tag="biasc")
            for t in range(T):
                nc.vector.memset(biasc[:, t:t + 1], float(t + 1))

            def load_h(t):
                ht = hp.tile([128, 32 * BH], f16, tag="hcur", name=f"h{t}")
                nc.gpsimd.dma_start(
                    ht.rearrange("p (jj c) -> p jj c", c=2 * BH),
                    stN[t, :, :, :].rearrange("jj p c -> p jj c"))
                hcur[t] = ht

            def build_masks(t):
                tl = []
                for g in range(NG):
                    sl = slice(g * NTPG * 128, (g + 1) * NTPG * 128)
                    m = mp.tile([128, NTPG * RPC], f16, tag="m")
                    nc.sync.dma_start(
                        m.rearrange("p (n m) -> p n m", n=NTPG),
                        maskT[t, sl, :].rearrange("(n p) m -> p n m", p=128))
                    tl.append(m)
                mtiles[t] = tl

            def build_c1(t):
                tiles = []
                xv = xall.rearrange("p (j t b) -> p j t b", t=T, b=B)
                hv = hcur[t].rearrange("p (j b f) -> p j b f", b=B, f=H)
                for g in range(NG):
                    c1 = cp.tile([128, F4], f16, tag="c1")
                    c1v = c1.rearrange("p (n b k) -> p n b k", b=B, k=H + 1)
                    nc.vector.tensor_copy(
                        c1v[:, :, :, 1:],
                        hv[:, g * NTPG:(g + 1) * NTPG, :, :])
                    nc.vector.tensor_copy(
                        c1v[:, :, :, 0:1],
                        xv[:, g * NTPG:(g + 1) * NTPG, t:t + 1, :]
                        .rearrange("p n o b -> p n b o"))
                    tiles.append(c1)
                return tiles

            def conv_mms(t, ctiles, za, zb):
                zh = [za, zb]
                for g in range(NG):
                    for ntl in range(NTPG):
                        first = g == 0 and ntl == 0
                        last = g == NG - 1 and ntl == NTPG - 1
                        for mt in range(MT):
                            nc.tensor.matmul(
                                zh[mt // 2][:, (mt % 2) * F1:(mt % 2 + 1) * F1],
                                mtiles[t][g][:, ntl * RPC + mt * 128:
                                             ntl * RPC + (mt + 1) * 128],
                                ctiles[g][:, ntl * F1:(ntl + 1) * F1],
                                start=(first and mt % 2 == 0),
                                stop=last)

            def small_path(t, mt, s1n):
                """S1[mt] -> gcn1 -> sigmoid((t+1)*b1 bias) -> sigp
                [p,(half,b,f)] fp16 (b1 is all-ones per the problem spec)."""
                g1 = g1p.tile([128, 2 * BH], f32, tag="g1")
                for b in range(B):
                    tz = tpzp.tile([H + 1, 128], f32, tag="tz")
                    nc.tensor.transpose(
                        tz[:], s1n[:, b * (H + 1):(b + 1) * (H + 1)], ident[:])
                    zbt = wk.tile([H + 1, 128], f32, tag="zbt")
                    nc.scalar.copy(zbt[:], tz[:])
                    nc.tensor.matmul(g1[:, b * 2 * H:(b + 1) * 2 * H],
                                     zbt[:], w1s[:], start=True, stop=True)
                sigp = wk.tile([128, 2 * BH], f16, tag="sigp")
                nc.scalar.activation(
                    sigp.rearrange("p (h b f) -> p b h f", h=2, b=B),
                    g1.rearrange("p (b h f) -> p b h f", h=2, b=B),
                    Act.Sigmoid, bias=biasc[:, t:t + 1])
                return sigp

            def shard_write(dst, mtl, sigp):
                """sigp [pp,(h,b,f)] -> p-major col-blocks 2*mtl, 2*mtl+1.

                Value for local hnode mtl*256 + 2*pp + h lands at
                row 2*pp2+h, col-block j2 = 2*mtl+hh, pp = hh*64+pp2."""
                for hh in range(2):
                    j2 = 2 * mtl + hh
                    nc.sync.dma_start(
                        dst[:, j2 * BH:(j2 + 1) * BH]
                        .rearrange("(pp h) c -> pp h c", h=2),
                        sigp[hh * 64:(hh + 1) * 64, :]
                        .rearrange("pp (h c) -> pp h c", h=2))

            def conv2_step(t):
                rls = []
                for r in range(NC):
                    rlr = rlp.tile([128, 4 * BH], f16, tag=f"rl{r}",
                                   name=f"rl{t}_{r}")
                    nc.scalar.dma_start(rlr[:], agdst[t][r, :, :])
                    rls.append(rlr)
                xv = xall.rearrange("p (j t b) -> p j t b", t=T, b=B)
                hv = hcur[t].rearrange("p (j b f) -> p j b f", b=B, f=H)
                ctiles = []
                for g in range(NG):
                    c2 = cp.tile([128, F4], f16, tag="c2")
                    c2v = c2.rearrange("p (n b k) -> p n b k", b=B, k=H + 1)
                    nc.vector.tensor_mul(
                        c2v[:, :, :, 1:],
                        rls[g].rearrange("p (j b f) -> p j b f", b=B, f=H),
                        hv[:, g * NTPG:(g + 1) * NTPG])
                    nc.vector.tensor_copy(
                        c2v[:, :, :, 0:1],
                        xv[:, g * NTPG:(g + 1) * NTPG, t:t + 1, :]
                        .rearrange("p n o b -> p n b o"))
                    ctiles.append(c2)
                z2a = z2p.tile([128, 2 * F1], f32, tag="z2a", name=f"z2a{t}")
                z2b = z2p.tile([128, 2 * F1], f32, tag="z2b", name=f"z2b{t}")
                conv_mms(t, ctiles, z2a, z2b)
                z2h = [z2a, z2b]
                for mt in range(MT):
                    s2n = accp.tile([128, F1], f32, tag=f"s2_{mt}")
                    nc.vector.tensor_add(
                        s2n[:], s2[mt][:],
                        z2h[mt // 2][:, (mt % 2) * F1:(mt % 2 + 1) * F1])
                    s2[mt] = s2n

            # ---------------- main loop (conv2 lag 2) ----------------
            for t in range(T):
                load_h(t)
                build_masks(t)
                if t == 0:
                    nc.sync.dma_start(w1s[:], w1h[:])
                    nc.sync.dma_start(w2s[:], w2h[:])
                if t >= 3:
                    conv2_step(t - 2)
                c1t = build_c1(t)
                z1a = z1p.tile([128, 2 * F1], f32, tag="z1a", name=f"z1a{t}")
                z1b = z1p.tile([128, 2 * F1], f32, tag="z1b", name=f"z1b{t}")
                conv_mms(t, c1t, z1a, z1b)
                z1h = [z1a, z1b]
                for mt in range(MT):
                    s1n = accp.tile([128, F1], f32, tag=f"s1_{mt}")
                    nc.vector.tensor_add(
                        s1n[:], s1[mt][:],
                        z1h[mt // 2][:, (mt % 2) * F1:(mt % 2 + 1) * F1])
                    s1[mt] = s1n
                if t == 2:
                    conv2_step(0)
                for mt in range(MT):
                    if mt < 2:
                        sigp = small_path(t, mt, s1[mt])
                        shard_write(agsrc[t], mt, sigp)
                    elif t == T - 1:
                        sigp = small_path(t, mt, s1[mt])
                        shard_write(usrc, mt - 2, sigp)
                nc.gpsimd.collective_compute(
                    "AllGather", Alu.bypass, replica_groups=[CORES],
                    ins=[agsrc[t][:]], outs=[agdst[t][:]])
                if t == T - 1:
                    nc.gpsimd.collective_compute(
                        "AllGather", Alu.bypass, replica_groups=[CORES],
                        ins=[usrc[:]], outs=[udst[:]])
            conv2_step(T - 2)
            conv2_step(T - 1)
            uall = rlp.tile([128, NC * 4 * BH], f16, tag="cu", name="uall")
            nc.sync.dma_start(
                uall.rearrange("p (r c) -> p r c", r=NC),
                udst.rearrange("r p c -> p r c"))

            # ---------------- tail: tanh -> cusrc; AG; output -------------
            for mt in range(MT):
                g2 = g1p.tile([128, BH], f32, tag="g1")
                for b in range(B):
                    tz = tpzp.tile([H + 1, 128], f32, tag="tz")
                    nc.tensor.transpose(
                        tz[:], s2[mt][:, b * (H + 1):(b + 1) * (H + 1)],
                        ident[:])
                    zbt = wk.tile([H + 1, 128], f32, tag="zbt")
                    nc.scalar.copy(zbt[:], tz[:])
                    nc.tensor.matmul(g2[:, b * H:(b + 1) * H],
                                     zbt[:], w2s[:], start=True, stop=True)
                tanb = wk.tile([128, BH], f16, tag="sigb")
                nc.scalar.activation(tanb[:], g2[:], Act.Tanh)
                # c values are keyed by own-node rows directly: p = pp
                nc.sync.dma_start(cusrc[:, mt * BH:(mt + 1) * BH], tanb[:])

            nc.gpsimd.collective_compute(
                "AllGather", Alu.bypass, replica_groups=[CORES],
                ins=[cusrc[:]], outs=[cudst[:]])

            # j-ordered c gather: rank r c-blocks {0,1}->j {2r,2r+1},
            # {2,3}->j {16+2r,16+2r+1}; alternate DMA rings
            clall = rlp.tile([128, 32 * BH], f16, tag="cu", name="clall")
            for r in range(NC):
                nc.sync.dma_start(
                    clall[:, (2 * r) * BH:(2 * r + 2) * BH],
                    cudst[r, :, 0:2 * BH])
                nc.scalar.dma_start(
                    clall[:, (16 + 2 * r) * BH:(16 + 2 * r + 2) * BH],
                    cudst[r, :, 2 * BH:4 * BH])

            # out = c + u*(h9 - c): 4-tile slabs (uall/clall are j-ordered)
            for g in range(NG):
                sl = slice(g * NTPG * BH, (g + 1) * NTPG * BH)
                outt = mp.tile([128, NTPG * BH], f32, tag="m", name=f"out{g}")
                dd = wk.tile([128, NTPG * BH], f16, tag="dd")
                nc.vector.tensor_sub(dd[:], hcur[T - 1][:, sl], clall[:, sl])
                mm = wk.tile([128, NTPG * BH], f16, tag="mmv")
                nc.vector.tensor_mul(mm[:], uall[:, sl], dd[:])
                nc.vector.tensor_add(outt[:], mm[:], clall[:, sl])
                nc.sync.dma_start(
                    houtN[g * NTPG * 128:(g + 1) * NTPG * 128, :]
                    .rearrange("(j p) c -> p j c", p=128),
                    outt.rearrange("p (j c) -> p j c", c=BH))

    nc.finalize()
    return nc


_NC_CACHE = None


def _get_nc():
    global _NC_CACHE
    if _NC_CACHE is None:
        _NC_CACHE = _build_nc()
    return _NC_CACHE


def make_in_maps(inputs, states, dtw, spec_lap, laplacian, time_delay,
                 W1, b1, W2, b2):
    f16n = np.float16
    eye = np.eye(N, dtype=np.float32)
    tdc = np.ceil(np.abs(time_delay.astype(np.float64))).astype(np.float32)
    sle = spec_lap + eye
    lap9 = ((2.0 / 3.0) * (dtw * (tdc > 0) + sle + laplacian)).astype(np.float32)
    # pre-masked per-step matrices, transposed: maskT[t] = M_t^T
    maskTs = np.empty((T, N, N), np.float16)
    for t in range(T - 1):
        maskTs[t] = (np.where(tdc > float(9 - t), dtw, 0.0) + sle).T.astype(f16n)
    maskTs[T - 1] = lap9.T.astype(f16n)
    # states: [T,B,N,H] -> [T, N, B, H] -> [T, jj(16), p(128), j2(2)*BH]
    st = states.reshape(T, B, N, H).transpose(0, 2, 1, 3)
    stN3 = np.ascontiguousarray(
        st.reshape(T, 16, 2, 128, B * H).transpose(0, 1, 3, 2, 4)
        .reshape(T, 16, 128, 2 * B * H)).astype(f16n)
    # x, p-major: xN[p, (j,t,b)] = inputs[b,t,j*128+p]
    xNh = np.ascontiguousarray(
        inputs.transpose(2, 1, 0).reshape(32, 128, T * B).transpose(1, 0, 2)
        .reshape(128, 32 * T * B)).astype(f16n)
    w1hv = (0.5 * W1).astype(np.float32)
    w2hv = (0.5 * W2).astype(np.float32)
    # biases are folded into the on-device activations as scalars
    assert np.allclose(b1, 1.0) and np.allclose(b2, 0.0), "spec fill changed"

    in_maps = []
    for c in range(NC):
        rc = np.concatenate([np.arange(c * HB, (c + 1) * HB),
                             2048 + np.arange(c * HB, (c + 1) * HB)])
        in_maps.append(dict(
            maskT=np.ascontiguousarray(maskTs[:, :, rc]),
            stN=stN3, xN=xNh, w1h=w1hv, w2h=w2hv,
        ))
    return in_maps


def kernel(inputs, states, dtw, spec_lap, laplacian, time_delay,
           W1, b1, W2, b2):
    in_maps = make_in_maps(
        np.asarray(inputs, np.float32), np.asarray(states, np.float32),
        np.asarray(dtw, np.float32), np.asarray(spec_lap, np.float32),
        np.asarray(laplacian, np.float32), np.asarray(time_delay, np.float32),
        np.asarray(W1, np.float32), np.asarray(b1, np.float32),
        np.asarray(W2, np.float32), np.asarray(b2, np.float32),
    )
    nc = _get_nc()
    res = run_bass_kernel_spmd(nc, in_maps, CORES, trace=False)
    out = np.asarray(res.results[0]["houtN"], np.float32)  # [N, B*H]
    return np.ascontiguousarray(
        out.reshape(N, B, H).transpose(1, 0, 2)).reshape(B, N * H)


# revision 31
# speedup vs baseline: 1.1480x; 1.1480x over previous
"""FESTGCN Trainium2 kernel v5: 8-core SPMD Bass/Tile implementation.

Algorithm (reference semantics, validated in sim_v2.py at ~5e-3):
  For t in 0..9:
    M_t = dtw * (ceil|td| > 9-t) + (spec_lap + I)       [t=9: host-folded
          (2/3)(dtw*(td>0) + spec_lap + I + laplacian), no mask]
    S1 += M_t^T-block @ c1_t      c1_t = [x_t | h_t]   (inputs only)
    r_t = sigmoid(0.5*S1@W1 + (t+1)b1) for r-half nodes (0..2047)
    S2 += M_t^T-block @ c2_t      c2_t = [x_t | r_t*h_t]
  u = sigmoid(...)[u-half] at t=9 ; c = tanh(0.5*S2@W2 + 10 b2)
  out = u*h_9 + (1-u)*c

Sharding: interleaved row blocks (core c owns nodes [c*256,(c+1)*256)
u [2048+c*256, ...)), so m-tiles 0,1 are r-nodes (per-step sigmoid +
AllGather payload) and 2,3 are u-nodes (sigmoid at t=9 only). conv1
depends only on inputs so it runs ahead; conv2 is emitted with a
2-step lag to hide AllGather latency; rl gathers ride the second
HWDGE ring (scalar) to avoid SP-FIFO head-of-line blocking.

Masks are host-precomputed fp16 and streamed from DRAM (memory
regime). AG payloads are partition-major ([p, j2-block, b, f], where
gathered hnode = (4*rank+j2)*128+p), so producer writes, collective
shards, and per-rank consumer reads are all contiguous-chunk DMAs.
"""

import numpy as np

import concourse.bacc as bacc
import concourse.mybir as mybir
import concourse.tile as tile
from concourse.bass_utils import run_bass_kernel_spmd

B, T, N, H = 4, 10, 4096, 32
NC = 8
HB = 256                 # nodes per half-block per core
RPC = 2 * HB             # 512 owned rows per core
NG = 8                   # contraction groups (4 n-tiles each)
NTPG = 4
MT = 4
F1 = B * (H + 1)         # 132
F4 = NTPG * F1           # 528
BH = B * H               # 128
f32 = mybir.dt.float32
f16 = mybir.dt.float16
Alu = mybir.AluOpType
Act = mybir.ActivationFunctionType
CORES = list(range(NC))


def _build_nc():
    nc = bacc.Bacc(
        "TRN2",
        target_bir_lowering=False,
        debug=False,
        enable_asserts=True,
        num_devices=NC,
    )
    maskT = nc.dram_tensor("maskT", [T, N, RPC], f16, kind="ExternalInput").ap()
    # states pre-gathered per step: [T, jj(16), p(128), j2(2)*BH]
    stN = nc.dram_tensor("stN", [T, 16, 128, 2 * BH], f16,
                         kind="ExternalInput").ap()
    xN = nc.dram_tensor("xN", [128, 32 * T * B], f16, kind="ExternalInput").ap()
    w1h = nc.dram_tensor("w1h", [H + 1, 2 * H], f32, kind="ExternalInput").ap()
    w2h = nc.dram_tensor("w2h", [H + 1, H], f32, kind="ExternalInput").ap()
    houtN = nc.dram_tensor("houtN", [N, BH], f32, kind="ExternalOutput").ap()

    with tile.TileContext(nc) as tc:
        with (
            tc.tile_pool(name="xp", bufs=1) as xp,
            tc.tile_pool(name="hp", bufs=4) as hp,
            tc.tile_pool(name="rlp", bufs=2) as rlp,
            tc.tile_pool(name="mp", bufs=26) as mp,
            tc.tile_pool(name="cp", bufs=8) as cp,
            tc.tile_pool(name="accp", bufs=2) as accp,
            tc.tile_pool(name="wk", bufs=2) as wk,
            tc.tile_pool(name="sm", bufs=1) as sm,
            tc.tile_pool(name="z1p", bufs=2, space="PSUM") as z1p,
            tc.tile_pool(name="z2p", bufs=1, space="PSUM") as z2p,
            tc.tile_pool(name="tpzp", bufs=1, space="PSUM") as tpzp,
            tc.tile_pool(name="g1p", bufs=1, space="PSUM") as g1p,
            tc.tile_pool(name="dramp", bufs=1, space="DRAM") as dramp,
        ):
            # partition-major r_t payloads: shard [p, j2(4)*BH]
            agsrc = [
                dramp.tile([128, 4 * BH], f16, tag=f"agsrc{t}", name=f"agsrc{t}")
                for t in range(T)
            ]
            agdst = [
                dramp.tile([NC, 128, 4 * BH], f16, tag=f"agdst{t}",
                           name=f"agdst{t}", addr_space="Shared")
                for t in range(T)
            ]
            usrc = dramp.tile([128, 4 * BH], f16, tag="usrc", name="usrc")
            udst = dramp.tile([NC, 128, 4 * BH], f16, tag="udst",
                              name="udst", addr_space="Shared")
            # c payload: [p, (c mt0..3)*BH]
            cusrc = dramp.tile([128, 4 * BH], f16, tag="cusrc", name="cusrc")
            cudst = dramp.tile([NC, 128, 4 * BH], f16, tag="cudst",
                               name="cudst", addr_space="Shared")

            # ---------------- prologue ----------------
            iota_i = wk.tile([128, 128], mybir.dt.int32, tag="iota", bufs=1)
            nc.gpsimd.iota(iota_i[:], pattern=[[1, 128]], base=0,
                           channel_multiplier=-1)
            ident = sm.tile([128, 128], f32, tag="ident")
            nc.vector.tensor_scalar(ident[:], iota_i[:], 0, None,
                                    op0=Alu.is_equal)

            xall = xp.tile([128, 32 * T * B], f16, tag="xall")
            nc.sync.dma_start(xall[:], xN[:])

            s1 = [accp.tile([128, F1], f32, tag=f"s1_{mt}", name=f"s1_{mt}")
                  for mt in range(MT)]
            s2 = [accp.tile([128, F1], f32, tag=f"s2_{mt}", name=f"s2_{mt}")
                  for mt in range(MT)]
            for mt in range(MT):
                nc.vector.memset(s1[mt][:], 0.0)
                nc.vector.memset(s2[mt][:], 0.0)

            hcur = [None] * T
            mtiles = [None] * T
            w1s = sm.tile([H + 1, 2 * H], f32, tag="w1s")
            w2s = sm.tile([H + 1, H], f32, tag="w2s")
            biasc = sm.tile([128, T], f32, tag="biasc")
            for t in range(T):
                nc.vector.memset(biasc[:, t:t + 1], float(t + 1))

            def load_h(t):
                ht = hp.tile([128, 32 * BH], f16, tag="hcur", name=f"h{t}")
                nc.sync.dma_start(
                    ht.rearrange("p (jj c) -> p jj c", c=2 * BH),
                    stN[t, :, :, :].rearrange("jj p c -> p jj c"))
                hcur[t] = ht

            def build_masks(t):
                tl = []
                for g in range(NG):
                    sl = slice(g * NTPG * 128, (g + 1) * NTPG * 128)
                    m = mp.tile([128, NTPG * RPC], f16, tag="m")
                    nc.sync.dma_start(
                        m.rearrange("p (n m) -> p n m", n=NTPG),
                        maskT[t, sl, :].rearrange("(n p) m -> p n m", p=128))
                    tl.append(m)
                mtiles[t] = tl

            def build_c1(t):
                tiles = []
                xv = xall.rearrange("p (j t b) -> p j t b", t=T, b=B)
                hv = hcur[t].rearrange("p (j b f) -> p j b f", b=B, f=H)
                for g in range(NG):
                    c1 = cp.tile([128, F4], f16, tag="c1")
                    c1v = c1.rearrange("p (n b k) -> p n b k", b=B, k=H + 1)
                    nc.vector.tensor_copy(
                        c1v[:, :, :, 1:],
                        hv[:, g * NTPG:(g + 1) * NTPG, :, :])
                    nc.vector.tensor_copy(
                        c1v[:, :, :, 0:1],
                        xv[:, g * NTPG:(g + 1) * NTPG, t:t + 1, :]
                        .rearrange("p n o b -> p n b o"))
                    tiles.append(c1)
                return tiles

            def conv_mms(t, ctiles, za, zb):
                zh = [za, zb]
                for g in range(NG):
                    for ntl in range(NTPG):
                        first = g == 0 and ntl == 0
                        last = g == NG - 1 and ntl == NTPG - 1
                        for mt in range(MT):
                            nc.tensor.matmul(
                                zh[mt // 2][:, (mt % 2) * F1:(mt % 2 + 1) * F1],
                                mtiles[t][g][:, ntl * RPC + mt * 128:
                                             ntl * RPC + (mt + 1) * 128],
                                ctiles[g][:, ntl * F1:(ntl + 1) * F1],
                                start=(first and mt % 2 == 0),
                                stop=last)

            def small_path(t, mt, s1n):
                """S1[mt] -> gcn1 -> sigmoid((t+1)*b1 bias) -> sigp
                [p,(half,b,f)] fp16 (b1 is all-ones per the problem spec)."""
                g1 = g1p.tile([128, 2 * BH], f32, tag="g1")
                for b in range(B):
                    tz = tpzp.tile([H + 1, 128], f32, tag="tz")
                    nc.tensor.transpose(
                        tz[:], s1n[:, b * (H + 1):(b + 1) * (H + 1)], ident[:])
                    zbt = wk.tile([H + 1, 128], f32, tag="zbt")
                    nc.scalar.copy(zbt[:], tz[:])
                    nc.tensor.matmul(g1[:, b * 2 * H:(b + 1) * 2 * H],
                                     zbt[:], w1s[:], start=True, stop=True)
                sigp = wk.tile([128, 2 * BH], f16, tag="sigp")
                nc.scalar.activation(
                    sigp.rearrange("p (h b f) -> p b h f", h=2, b=B),
                    g1.rearrange("p (b h f) -> p b h f", h=2, b=B),
                    Act.Sigmoid, bias=biasc[:, t:t + 1])
                return sigp

            def shard_write(dst, mtl, sigp):
                """sigp [pp,(h,b,f)] -> p-major col-blocks 2*mtl, 2*mtl+1.

                Value for local hnode mtl*256 + 2*pp + h lands at
                row 2*pp2+h, col-block j2 = 2*mtl+hh, pp = hh*64+pp2."""
                for hh in range(2):
                    j2 = 2 * mtl + hh
                    nc.sync.dma_start(
                        dst[:, j2 * BH:(j2 + 1) * BH]
                        .rearrange("(pp h) c -> pp h c", h=2),
                        sigp[hh * 64:(hh + 1) * 64, :]
                        .rearrange("pp (h c) -> pp h c", h=2))

            def conv2_step(t):
                rls = []
                for r in range(NC):
                    rlr = rlp.tile([128, 4 * BH], f16, tag=f"rl{r}",
                                   name=f"rl{t}_{r}")
                    nc.scalar.dma_start(rlr[:], agdst[t][r, :, :])
                    rls.append(rlr)
                xv = xall.rearrange("p (j t b) -> p j t b", t=T, b=B)
                hv = hcur[t].rearrange("p (j b f) -> p j b f", b=B, f=H)
                ctiles = []
                for g in range(NG):
                    c2 = cp.tile([128, F4], f16, tag="c2")
                    c2v = c2.rearrange("p (n b k) -> p n b k", b=B, k=H + 1)
                    nc.vector.tensor_mul(
                        c2v[:, :, :, 1:],
                        rls[g].rearrange("p (j b f) -> p j b f", b=B, f=H),
                        hv[:, g * NTPG:(g + 1) * NTPG])
                    nc.vector.tensor_copy(
                        c2v[:, :, :, 0:1],
                        xv[:, g * NTPG:(g + 1) * NTPG, t:t + 1, :]
                        .rearrange("p n o b -> p n b o"))
                    ctiles.append(c2)
                z2a = z2p.tile([128, 2 * F1], f32, tag="z2a", name=f"z2a{t}")
                z2b = z2p.tile([128, 2 * F1], f32, tag="z2b", name=f"z2b{t}")
                conv_mms(t, ctiles, z2a, z2b)
                z2h = [z2a, z2b]
                for mt in range(MT):
                    s2n = accp.tile([128, F1], f32, tag=f"s2_{mt}")
                    nc.vector.tensor_add(
                        s2n[:], s2[mt][:],
                        z2h[mt // 2][:, (mt % 2) * F1:(mt % 2 + 1) * F1])
                    s2[mt] = s2n

            # ---------------- main loop (conv2 lag 2) ----------------
            for t in range(T):
                load_h(t)
                build_masks(t)
                if t == 0:
                    nc.sync.dma_start(w1s[:], w1h[:])
                    nc.sync.dma_start(w2s[:], w2h[:])
                if t >= 3:
                    conv2_step(t - 2)
                c1t = build_c1(t)
                z1a = z1p.tile([128, 2 * F1], f32, tag="z1a", name=f"z1a{t}")
                z1b = z1p.tile([128, 2 * F1], f32, tag="z1b", name=f"z1b{t}")
                conv_mms(t, c1t, z1a, z1b)
                z1h = [z1a, z1b]
                for mt in range(MT):
                    s1n = accp.tile([128, F1], f32, tag=f"s1_{mt}")
                    nc.vector.tensor_add(
                        s1n[:], s1[mt][:],
                        z1h[mt // 2][:, (mt % 2) * F1:(mt % 2 + 1) * F1])
                    s1[mt] = s1n
                if t == 2:
                    conv2_step(0)
                for mt in range(MT):
                    if mt < 2:
                        sigp = small_path(t, mt, s1[mt])
                        shard_write(agsrc[t], mt, sigp)
                    elif t == T - 1:
                        sigp = small_path(t, mt, s1[mt])
                        shard_write(usrc, mt - 2, sigp)
                nc.gpsimd.collective_compute(
                    "AllGather", Alu.bypass, replica_groups=[CORES],
                    ins=[agsrc[t][:]], outs=[agdst[t][:]])
                if t == T - 1:
                    nc.gpsimd.collective_compute(
                        "AllGather", Alu.bypass, replica_groups=[CORES],
                        ins=[usrc[:]], outs=[udst[:]])
            conv2_step(T - 2)
            conv2_step(T - 1)
            uall = rlp.tile([128, NC * 4 * BH], f16, tag="cu", name="uall")
            nc.sync.dma_start(
                uall.rearrange("p (r c) -> p r c", r=NC),
                udst.rearrange("r p c -> p r c"))

            # ---------------- tail: tanh -> cusrc; AG; output -------------
            for mt in range(MT):
                g2 = g1p.tile([128, BH], f32, tag="g1")
                for b in range(B):
                    tz = tpzp.tile([H + 1, 128], f32, tag="tz")
                    nc.tensor.transpose(
                        tz[:], s2[mt][:, b * (H + 1):(b + 1) * (H + 1)],
                        ident[:])
                    zbt = wk.tile([H + 1, 128], f32, tag="zbt")
                    nc.scalar.copy(zbt[:], tz[:])
                    nc.tensor.matmul(g2[:, b * H:(b + 1) * H],
                                     zbt[:], w2s[:], start=True, stop=True)
                tanb = wk.tile([128, BH], f16, tag="sigb")
                nc.scalar.activation(tanb[:], g2[:], Act.Tanh)
                # c values are keyed by own-node rows directly: p = pp
                nc.sync.dma_start(cusrc[:, mt * BH:(mt + 1) * BH], tanb[:])

            nc.gpsimd.collective_compute(
                "AllGather", Alu.bypass, replica_groups=[CORES],
                ins=[cusrc[:]], outs=[cudst[:]])

            # j-ordered c gather: rank r c-blocks {0,1}->j {2r,2r+1},
            # {2,3}->j {16+2r,16+2r+1}; alternate DMA rings
            clall = rlp.tile([128, 32 * BH], f16, tag="cu", name="clall")
            for r in range(NC):
                nc.sync.dma_start(
                    clall[:, (2 * r) * BH:(2 * r + 2) * BH],
                    cudst[r, :, 0:2 * BH])
                nc.scalar.dma_start(
                    clall[:, (16 + 2 * r) * BH:(16 + 2 * r + 2) * BH],
                    cudst[r, :, 2 * BH:4 * BH])

            # out = c + u*(h9 - c): 4-tile slabs (uall/clall are j-ordered)
            for g in range(NG):
                sl = slice(g * NTPG * BH, (g + 1) * NTPG * BH)
                outt = mp.tile([128, NTPG * BH], f32, tag="m", name=f"out{g}")
                dd = wk.tile([128, NTPG * BH], f16, tag="dd")
                nc.vector.tensor_sub(dd[:], hcur[T - 1][:, sl], clall[:, sl])
                mm = wk.tile([128, NTPG * BH], f16, tag="mmv")
                nc.vector.tensor_mul(mm[:], uall[:, sl], dd[:])
                nc.vector.tensor_add(outt[:], mm[:], clall[:, sl])
                nc.sync.dma_start(
                    houtN[g * NTPG * 128:(g + 1) * NTPG * 128, :]
                    .rearrange("(j p) c -> p j c", p=128),
                    outt.rearrange("p (j c) -> p j c", c=BH))

    nc.finalize()
    return nc


_NC_CACHE = None


def _get_nc():
    global _NC_CACHE
    if _NC_CACHE is None:
        _NC_CACHE = _build_nc()
    return _NC_CACHE


def make_in_maps(inputs, states, dtw, spec_lap, laplacian, time_delay,
                 W1, b1, W2, b2):
    f16n = np.float16
    eye = np.eye(N, dtype=np.float32)
    tdc = np.ceil(np.abs(time_delay.astype(np.float64))).astype(np.float32)
    sle = spec_lap + eye
    lap9 = ((2.0 / 3.0) * (dtw * (tdc > 0) + sle + laplacian)).astype(np.float32)
    # pre-masked per-step matrices, transposed: maskT[t] = M_t^T
    maskTs = np.empty((T, N, N), np.float16)
    for t in range(T - 1):
        maskTs[t] = (np.where(tdc > float(9 - t), dtw, 0.0) + sle).T.astype(f16n)
    maskTs[T - 1] = lap9.T.astype(f16n)
    # states: [T,B,N,H] -> [T, N, B, H] -> [T, jj(16), p(128), j2(2)*BH]
    st = states.reshape(T, B, N, H).transpose(0, 2, 1, 3)
    stN3 = np.ascontiguousarray(
        st.reshape(T, 16, 2, 128, B * H).transpose(0, 1, 3, 2, 4)
        .reshape(T, 16, 128, 2 * B * H)).astype(f16n)
    # x, p-major: xN[p, (j,t,b)] = inputs[b,t,j*128+p]
    xNh = np.ascontiguousarray(
        inputs.transpose(2, 1, 0).reshape(32, 128, T * B).transpose(1, 0, 2)
        .reshape(128, 32 * T * B)).astype(f16n)
    w1hv = (0.5 * W1).astype(np.float32)
    w2hv = (0.5 * W2).astype(np.float32)
    # biases are folded into the on-device activations as scalars
    assert np.allclose(b1, 1.0) and np.allclose(b2, 0.0), "spec fill changed"

    in_maps = []
    for c in range(NC):
        rc = np.concatenate([np.arange(c * HB, (c + 1) * HB),
                             2048 + np.arange(c * HB, (c + 1) * HB)])
        in_maps.append(dict(
            maskT=np.ascontiguousarray(maskTs[:, :, rc]),
            stN=stN3, xN=xNh, w1h=w1hv, w2h=w2hv,
        ))
    return in_maps


def kernel(inputs, states, dtw, spec_lap, laplacian, time_delay,
           W1, b1, W2, b2):
    in_maps = make_in_maps(
        np.asarray(inputs, np.float32), np.asarray(states, np.float32),
        np.asarray(dtw, np.float32), np.asarray(spec_lap, np.float32),
        np.asarray(laplacian, np.float32), np.asarray(time_delay, np.float32),
        np.asarray(W1, np.float32), np.asarray(b1, np.float32),
        np.asarray(W2, np.float32), np.asarray(b2, np.float32),
    )
    nc = _get_nc()
    res = run_bass_kernel_spmd(nc, in_maps, CORES, trace=False)
    out = np.asarray(res.results[0]["houtN"], np.float32)  # [N, B*H]
    return np.ascontiguousarray(
        out.reshape(N, B, H).transpose(1, 0, 2)).reshape(B, N * H)
